# revision 33
# baseline (speedup 1.0000x reference)
"""Multi-head causal self-attention with RoPE, tensor-parallel over heads
across 8 Trainium2 NeuronCores.

Strategy (Megatron-style TP over heads), v4:
  - Each core owns 2 of the 16 heads: rows [c*256,(c+1)*256) of Wq/Wk/Wv
    and the matching columns of Wo. Host sums the 8 partial outputs.
  - q/k projections run in fp8e4m3 with DoubleRow perf mode (0.5
    cycles/row, 2x PE throughput); the host pre-scales Wq/Wk by 64 to
    center the fp8 dynamic range, and the 64*64 factor is folded into
    the exp scale. The q/k quantization noise washes out through the
    softmax (diffuse attention); the value path (v, Wo) stays bf16.
  - Everything else in bf16 (PSUM accumulation fp32).
  - All DRAM operands are host-prearranged so every DMA is contiguous
    per partition (128 descriptors, cheap HWDGE issue); out is written
    oc-major [16, 128, bs] and reassembled on host.
  - Batch-pipelined generator emission: p1(b0) -> [p1(b1) || attn(b0)]
    -> [attn(b1) || p3(ready slabs)] -> p3(last slab). Independent
    matmuls fill the in-order PE queue between each score-matmul ->
    exp -> AV-matmul dependency chain.
  - Causal: off-diagonal k-chunks full-width; the 4 diagonal chunks of
    each 512-wide q-chunk use a reduced column range plus a 128x128
    additive triangle mask.
"""

import sys

import numpy as np

B, S, DIM = 2, 2048, 2048
NUM_HEADS = 16
HD = 128
N_CORES = 8
HPC = NUM_HEADS // N_CORES  # heads per core
DLOC = HPC * HD             # per-core slice of the model dim
ROPE_BASE = 10000.0
NDIN = DIM // 128           # contraction chunks for projections
SC1 = 512                   # phase-1 s-chunk
QCH = 512                   # attention q-chunk
NQC = S // QCH              # q-chunks per batch
NS1B = S // SC1             # phase-1 s-chunks per batch
WSCALE = 64.0               # fp8 pre-scale on Wq/Wk

_PROGRAM_CACHE = {}


def _rope_tables_T(seq_len, head_dim):
    # match reference float32 arithmetic: inv_freq over even indices,
    # emb = cat(freqs, freqs); returned transposed [head_dim, seq_len]
    inv_freq = (
        1.0
        / (np.float32(ROPE_BASE)
           ** (np.arange(0, head_dim, 2, dtype=np.float32) / np.float32(head_dim)))
    ).astype(np.float32)
    t = np.arange(seq_len, dtype=np.float32)
    freqs = np.outer(t, inv_freq).astype(np.float32)      # [S, D/2]
    emb = np.concatenate([freqs, freqs], axis=-1)         # [S, D]
    return (
        np.ascontiguousarray(np.cos(emb).astype(np.float32).T),
        np.ascontiguousarray(np.sin(emb).astype(np.float32).T),
    )


def _rot_matrix_T(head_dim):
    # rotated = cat(-x[1::2], x[::2]) = R @ x; return R.T [D, D]
    d2 = head_dim // 2
    R = np.zeros((head_dim, head_dim), dtype=np.float32)
    for dp in range(d2):
        R[dp, 2 * dp + 1] = -1.0
    for dp in range(d2, head_dim):
        R[dp, 2 * (dp - d2)] = 1.0
    return np.ascontiguousarray(R.T)


def _tri_mask():
    # tri[kk, qq] = 0 if kk <= qq else -1e9 (additive, pre-exp-scale)
    kk = np.arange(128)[:, None]
    qq = np.arange(128)[None, :]
    return np.where(kk <= qq, 0.0, -1e9).astype(np.float32)


def _run_interleaved(*specs):
    """specs: (generator, steps_per_round). Round-robin until exhausted."""
    live = [[iter(g), k] for g, k in specs]
    while live:
        for item in list(live):
            g, k = item
            try:
                for _ in range(k):
                    next(g)
            except StopIteration:
                live.remove(item)


def build_program(b=B, s=S, dim=DIM):
    """Builds the per-core SPMD Bass program (identical on every core)."""
    if "/opt/trn_rl_repo" not in sys.path:
        sys.path.insert(0, "/opt/trn_rl_repo")
    import concourse.bacc as bacc
    import concourse.mybir as mybir
    import concourse.tile as tile

    f32 = mybir.dt.float32
    bf16 = mybir.dt.bfloat16
    fp8 = mybir.dt.float8e4
    EXP = mybir.ActivationFunctionType.Exp
    DR = mybir.MatmulPerfMode.DoubleRow

    bs = b * s
    scale = float(HD) ** -0.5 / (WSCALE * WSCALE)
    nsc = s // 128   # 128-token chunks per batch
    ngc = bs // SC1  # global 512-token chunks

    nc = bacc.Bacc("TRN2", target_bir_lowering=False, debug=False)

    # host-prearranged layouts: contiguous per partition
    xP_d = nc.dram_tensor("xP", [ngc, 128, NDIN * SC1], bf16, kind="ExternalInput")
    x8P_d = nc.dram_tensor("x8P", [ngc, 128, NDIN * SC1], fp8, kind="ExternalInput")
    wq8_d = nc.dram_tensor("wq8", [128, NDIN * DLOC], fp8, kind="ExternalInput")
    wk8_d = nc.dram_tensor("wk8", [128, NDIN * DLOC], fp8, kind="ExternalInput")
    wqb_d = nc.dram_tensor("wqb", [128, NDIN * DLOC], bf16, kind="ExternalInput")
    wkb_d = nc.dram_tensor("wkb", [128, NDIN * DLOC], bf16, kind="ExternalInput")
    wvP_d = nc.dram_tensor("wvP", [128, NDIN * DLOC], bf16, kind="ExternalInput")
    woP_d = nc.dram_tensor("woP", [128, HPC * dim], bf16, kind="ExternalInput")
    cosT_d = nc.dram_tensor("cosT", [HD, bs], bf16, kind="ExternalInput")
    sinT_d = nc.dram_tensor("sinT", [HD, bs], bf16, kind="ExternalInput")
    rT_d = nc.dram_tensor("rT", [HD, HD], bf16, kind="ExternalInput")
    ones_d = nc.dram_tensor("ones", [HD, HD], bf16, kind="ExternalInput")
    tri_d = nc.dram_tensor("tri", [HD, HD], bf16, kind="ExternalInput")
    out_d = nc.dram_tensor("out", [dim // 128, 128, bs], bf16, kind="ExternalOutput")

    with tile.TileContext(nc) as tc:
        with (
            tc.tile_pool(name="persist", bufs=1) as persist,
            tc.tile_pool(name="work", bufs=2) as work,
            tc.tile_pool(name="ptp", bufs=4) as ptp,
            tc.tile_pool(name="otp", bufs=3) as otp,
            tc.tile_pool(name="ps_at", bufs=1, space="PSUM") as ps_at,
        ):
            # ---- persistent tiles ----
            qT = persist.tile([128, HPC, bs], bf16)   # roped q (x64), [d, h, tok]
            kT = persist.tile([128, HPC, bs], bf16)
            vS = persist.tile([128, bs // 128, DLOC], bf16)  # [tok, chunk, d]
            uT = persist.tile([128, HPC, bs], bf16)   # attention out
            wq_s = persist.tile([128, NDIN, DLOC], fp8)
            wk_s = persist.tile([128, NDIN, DLOC], fp8)
            # bf16 copies for the first 512 tokens of each batch: early
            # causal rows have concentrated attention, so fp8 q/k noise
            # doesn't wash out there (x64-scaled like the fp8 path)
            wqb_s = persist.tile([128, NDIN, DLOC], bf16)
            wkb_s = persist.tile([128, NDIN, DLOC], bf16)
            wv_s = persist.tile([128, NDIN, DLOC], bf16)
            woT_s = persist.tile([128, HPC, dim], bf16)
            cosS = persist.tile([128, bs], bf16)
            sinS = persist.tile([128, bs], bf16)
            rTs = persist.tile([HD, HD], bf16)
            onesS = persist.tile([HD, HD], bf16)
            triS = persist.tile([HD, HD], bf16)

            # ---- startup DMAs: critical first, split across queues ----
            # first chain (si=0, bf16 path) needs wqb + xt interleaved in
            # consumption order on sync; everything else on scalar/gpsimd
            def wview(wd):
                return wd.rearrange("p (c m) -> p c m", c=NDIN)

            nc.sync.dma_start(out=wqb_s[:, 0:8, :], in_=wview(wqb_d)[:, 0:8, :])
            nc.scalar.dma_start(out=wkb_s, in_=wview(wkb_d))
            nc.gpsimd.dma_start(out=rTs, in_=rT_d[:])
            nc.gpsimd.dma_start(out=onesS, in_=ones_d[:])
            nc.gpsimd.dma_start(out=triS, in_=tri_d[:])

            def startup_x0(xt, xsrc):
                # split the first x chunk across the sync and gpsimd queues
                # so the first bf16 qk chain unblocks ~2x sooner
                nc.sync.dma_start(out=xt[:, 0:4, :], in_=xsrc[:, 0:4, :])
                nc.gpsimd.dma_start(out=xt[:, 8:12, :], in_=xsrc[:, 8:12, :])
                nc.sync.dma_start(out=wqb_s[:, 8:16, :], in_=wview(wqb_d)[:, 8:16, :])
                nc.sync.dma_start(out=xt[:, 4:8, :], in_=xsrc[:, 4:8, :])
                nc.gpsimd.dma_start(out=xt[:, 12:16, :], in_=xsrc[:, 12:16, :])

            def startup_rest():
                # nothing PE-side in seg1 depends on cos/sin (RoPE runs on
                # DVE and only feeds the qT/kT stores read in seg2), so the
                # fp8 weights can jump the queue
                nc.scalar.dma_start(out=wv_s, in_=wview(wvP_d))
                nc.scalar.dma_start(out=wq_s, in_=wview(wq8_d))
                nc.scalar.dma_start(out=wk_s, in_=wview(wk8_d))
                nc.scalar.dma_start(out=cosS, in_=cosT_d[:])
                nc.scalar.dma_start(out=sinS, in_=sinT_d[:])
                # phase-3 weights: needed only after attn(b0); issue last
                nc.scalar.dma_start(
                    out=woT_s, in_=woP_d.rearrange("p (h n) -> p h n", h=HPC)
                )

            with (
                tc.tile_pool(name="p1x", bufs=2) as p1x,
                tc.tile_pool(name="p1ps", bufs=1, space="PSUM") as p1ps,
            ):
                # ------------- phase 1: qkv projections + RoPE -------------
                def p1_gen(bi, first=False):
                    for si in range(NS1B):
                        s0 = bi * s + si * SC1
                        g = s0 // SC1
                        xt = p1x.tile([128, NDIN, SC1], bf16, tag="xt")
                        xsrc = xP_d[g].rearrange("p (c t) -> p c t", c=NDIN)
                        if si == 0:
                            # bf16 q/k path for the first 512 tokens
                            xt8 = None
                            if first:
                                startup_x0(xt, xsrc)
                                startup_rest()
                            else:
                                nc.sync.dma_start(out=xt[:, :8, :], in_=xsrc[:, :8, :])
                                nc.sync.dma_start(out=xt[:, 8:, :], in_=xsrc[:, 8:, :])
                        else:
                            # split x traffic across the sync and gpsimd
                            # queues: the fp8 chains consume a chunk every
                            # ~11us but a lone queue needs ~9us for 3MB
                            xt8 = p1x.tile([128, NDIN, SC1], fp8, tag="xt8",
                                           bufs=3)
                            nc.sync.dma_start(
                                out=xt8,
                                in_=x8P_d[g].rearrange("p (c t) -> p c t", c=NDIN),
                            )
                            nc.sync.dma_start(out=xt[:, :8, :], in_=xsrc[:, :8, :])
                            nc.gpsimd.dma_start(out=xt[:, 8:, :], in_=xsrc[:, 8:, :])
                        yield
                        vq = [0, 1, 2, 3]

                        def v_chain(sub):
                            vacc = p1ps.tile([128, DLOC], f32, tag="v", bufs=1)
                            for c in range(NDIN):
                                nc.tensor.matmul(
                                    vacc,
                                    lhsT=xt[:, c, sub * 128:(sub + 1) * 128],
                                    rhs=wv_s[:, c, :],
                                    start=(c == 0),
                                    stop=(c == NDIN - 1),
                                )
                                if c % 4 == 3:
                                    yield
                            nc.scalar.copy(vS[:, s0 // 128 + sub, :], vacc)
                            yield

                        ci = 0
                        wpairs = (
                            ((wqb_s, qT), (wkb_s, kT)) if si == 0
                            else ((wq_s, qT), (wk_s, kT))
                        )
                        for w_t, store in wpairs:
                            for h in range(HPC):
                                acc = p1ps.tile([128, SC1], f32, tag="qk", bufs=2)
                                if si == 0:
                                    for c in range(NDIN):
                                        nc.tensor.matmul(
                                            acc,
                                            lhsT=w_t[:, c, h * HD:(h + 1) * HD],
                                            rhs=xt[:, c, :],
                                            start=(c == 0),
                                            stop=(c == NDIN - 1),
                                        )
                                        if c % 4 == 3:
                                            yield
                                else:
                                    for c2 in range(0, NDIN, 2):
                                        nc.tensor.matmul(
                                            acc,
                                            lhsT=w_t[:, c2:c2 + 2, h * HD:(h + 1) * HD],
                                            rhs=xt8[:, c2:c2 + 2, :],
                                            start=(c2 == 0),
                                            stop=(c2 == NDIN - 2),
                                            perf_mode=DR,
                                        )
                                        if c2 % 4 == 2:
                                            yield
                                raw = p1x.tile([128, SC1], bf16, tag="raw")
                                nc.vector.tensor_copy(raw, acc)
                                rot = p1ps.tile([128, SC1], f32, tag="rot", bufs=1)
                                nc.tensor.matmul(
                                    rot, lhsT=rTs, rhs=raw, start=True, stop=True
                                )
                                yield
                                t1 = p1x.tile([128, SC1], bf16, tag="t1")
                                nc.vector.tensor_mul(t1, raw, cosS[:, s0:s0 + SC1])
                                t2 = p1x.tile([128, SC1], bf16, tag="t2")
                                nc.vector.tensor_mul(t2, rot, sinS[:, s0:s0 + SC1])
                                nc.vector.tensor_add(
                                    store[:, h, s0:s0 + SC1], t1, t2
                                )
                                yield
                                ci += 1
                                # delay v chains past the first two qk chains
                                # (wv lands late in the startup order)
                                if vq and ci >= 2:
                                    yield from v_chain(vq.pop(0))
                        while vq:
                            yield from v_chain(vq.pop(0))

                # ------------- phase 2: causal attention -------------
                # p3q: output-projection slabs whose uT columns are complete
                p3q = []

                def attn_gen(bi):
                    # NOTE: pt must stay bf16 — real scores reach 9.08, so
                    # exp hits 8.8e3, far over fp8-e4m3's 448 max (NaN)
                    for qc in range(NQC):
                        for h in range(HPC):
                            q0 = bi * s + qc * QCH
                            nkc = (qc + 1) * QCH // 128
                            outp = ps_at.tile([128, QCH], f32, tag="o", bufs=1)
                            lrep = ps_at.tile([128, QCH], f32, tag="l", bufs=1)
                            npair = (nkc - 4) // 2
                            nones = npair + 4
                            li = 0
                            prev_pt = None
                            for kc in range(nkc):
                                k0 = bi * s + kc * 128
                                di = kc - (nkc - 4)
                                lo = max(0, di * 128)
                                st = ps_at.tile([128, QCH], f32, tag="st", bufs=2)
                                nc.tensor.matmul(
                                    st[:, lo:],
                                    lhsT=kT[:, h, k0:k0 + 128],
                                    rhs=qT[:, h, q0 + lo:q0 + QCH],
                                    start=True,
                                    stop=True,
                                )
                                if di >= 0:
                                    nc.vector.tensor_add(
                                        st[:, lo:lo + 128], st[:, lo:lo + 128], triS
                                    )
                                pt = ptp.tile([128, QCH], bf16, tag="pt")
                                nc.scalar.activation(
                                    pt[:, lo:], st[:, lo:], EXP, scale=scale
                                )
                                yield
                                nc.tensor.matmul(
                                    outp[:, lo:],
                                    lhsT=vS[:, bi * nsc + kc, h * HD:(h + 1) * HD],
                                    rhs=pt[:, lo:],
                                    start=(kc == 0),
                                    stop=(kc == nkc - 1),
                                )
                                if di >= 0:
                                    # diagonal chunks: single range-reduced
                                    # denominator matmul
                                    nc.tensor.matmul(
                                        lrep[:, lo:],
                                        lhsT=onesS,
                                        rhs=pt[:, lo:],
                                        start=(li == 0),
                                        stop=(li == nones - 1),
                                    )
                                    li += 1
                                elif kc % 2 == 1:
                                    # off-diagonal: pair-sum on DVE/GpSimd,
                                    # then one denominator matmul per pair
                                    lp = work.tile([128, QCH], bf16, tag="lp")
                                    eng = nc.vector if li % 2 == 0 else nc.gpsimd
                                    eng.tensor_add(lp, prev_pt, pt)
                                    nc.tensor.matmul(
                                        lrep,
                                        lhsT=onesS,
                                        rhs=lp,
                                        start=(li == 0),
                                        stop=(li == nones - 1),
                                    )
                                    li += 1
                                prev_pt = pt
                                yield
                            rec = work.tile([128, QCH], f32, tag="rec")
                            nc.vector.reciprocal_approx_fast(rec, lrep)
                            nc.vector.tensor_mul(
                                uT[:, h, q0:q0 + QCH], outp, rec
                            )
                            yield
                        p3q.append((bi, qc))

                # seg1: projections for batch 0 alone
                _run_interleaved((p1_gen(0, first=True), 1))
                # seg2: projections(b1) interleaved with attention(b0)
                _run_interleaved((p1_gen(1), 1), (attn_gen(0), 1))

            # p1 PSUM pools closed; banks free for phase 3
            with tc.tile_pool(name="ps3", bufs=2, space="PSUM") as ps3:
                # ------------- phase 3: output projection -------------
                # consumes ready 512-column slabs from p3q; a slab (bi, qc)
                # is pushed once both heads of that q-chunk wrote uT

                def p3_slab(bi, qc, ceng):
                    c0 = bi * s + qc * QCH
                    for oc in range(dim // 128):
                        pos = ps3.tile([128, QCH], f32, tag="p3", bufs=4)
                        for h in range(HPC):
                            nc.tensor.matmul(
                                pos,
                                lhsT=woT_s[:, h, oc * 128:(oc + 1) * 128],
                                rhs=uT[:, h, c0:c0 + QCH],
                                start=(h == 0),
                                stop=(h == HPC - 1),
                            )
                        ot = otp.tile([128, QCH], bf16, tag="ot", bufs=6)
                        eng = ceng[oc % len(ceng)]
                        if eng is nc.scalar:
                            eng.copy(ot, pos)
                        else:
                            eng.tensor_copy(ot, pos)
                        yield
                        # split write issue across the sync and gpsimd queues
                        deng = nc.sync if oc % 2 == 0 else nc.gpsimd
                        deng.dma_start(out=out_d[oc, :, c0:c0 + QCH], in_=ot)
                        yield

                def p3_consumer(n_slabs, ceng):
                    served = 0
                    while served < n_slabs:
                        if not p3q:
                            yield  # waiting on attention progress
                            continue
                        bi, qc = p3q.pop(0)
                        yield from p3_slab(bi, qc, ceng)
                        served += 1

                # seg3: attention(b1) interleaved with all ready output-
                # projection slabs (b0 now, b1 as q-chunks complete);
                # GpSimd cannot access PSUM, and Act owns the exps, so
                # copies lean on DVE
                _run_interleaved(
                    (attn_gen(1), 1),
                    (p3_consumer(2 * NQC - 1, [nc.vector, nc.vector, nc.scalar]), 1),
                )
                # tail: the final slab has nothing left to overlap with;
                # alternate copies over the now-idle Act and DVE
                _run_interleaved((p3_consumer(1, [nc.scalar, nc.vector]), 1))

    nc.compile()
    return nc


def make_in_maps(x, Wq, Wk, Wv, Wo, b=B, s=S, dim=DIM, n_cores=N_CORES):
    import ml_dtypes

    bf = ml_dtypes.bfloat16
    f8 = ml_dtypes.float8_e4m3fn
    bs = b * s
    xT = np.ascontiguousarray(x.reshape(bs, dim).T)          # [dim, bs] f32
    # [g, p, c*SC1+t]: per-512-token chunk, contiguous per partition
    xP4 = np.ascontiguousarray(
        xT.reshape(NDIN, 128, bs // SC1, SC1).transpose(2, 1, 0, 3)
    ).reshape(bs // SC1, 128, NDIN * SC1)
    xP = xP4.astype(bf)
    x8P = xP4.astype(f8)
    cosT1, sinT1 = _rope_tables_T(s, HD)
    cosT = np.ascontiguousarray(np.tile(cosT1, (1, b))).astype(bf)
    sinT = np.ascontiguousarray(np.tile(sinT1, (1, b))).astype(bf)
    rT = _rot_matrix_T(HD).astype(bf)
    ones = np.ones((HD, HD), dtype=np.float32).astype(bf)
    tri = _tri_mask().astype(bf)

    def wprep(wT):  # [dim, DLOC] -> [128, NDIN*DLOC] contiguous rows
        return np.ascontiguousarray(
            wT.reshape(NDIN, 128, DLOC).transpose(1, 0, 2)
        ).reshape(128, NDIN * DLOC)

    in_maps = []
    for c in range(n_cores):
        sl = slice(c * DLOC, (c + 1) * DLOC)
        woT = np.ascontiguousarray(Wo[:, sl].T)  # [DLOC, dim]
        woP = np.ascontiguousarray(
            woT.reshape(HPC, 128, dim).transpose(1, 0, 2)
        ).reshape(128, HPC * dim)
        in_maps.append(
            {
                "xP": xP,
                "x8P": x8P,
                "wq8": wprep(
                    np.ascontiguousarray(Wq[sl, :].T) * WSCALE
                ).astype(f8),
                "wk8": wprep(
                    np.ascontiguousarray(Wk[sl, :].T) * WSCALE
                ).astype(f8),
                "wqb": wprep(
                    np.ascontiguousarray(Wq[sl, :].T) * WSCALE
                ).astype(bf),
                "wkb": wprep(
                    np.ascontiguousarray(Wk[sl, :].T) * WSCALE
                ).astype(bf),
                "wvP": wprep(np.ascontiguousarray(Wv[sl, :].T)).astype(bf),
                "woP": woP.astype(bf),
                "cosT": cosT,
                "sinT": sinT,
                "rT": rT,
                "ones": ones,
                "tri": tri,
            }
        )
    return in_maps


def kernel(x, Wq, Wk, Wv, Wo, _trace=False):
    """Full-input / full-output entry point. Shards over 8 cores internally."""
    if "/opt/trn_rl_repo" not in sys.path:
        sys.path.insert(0, "/opt/trn_rl_repo")
    from concourse.bass_utils import run_bass_kernel_spmd

    x = np.asarray(x, dtype=np.float32)
    Wq, Wk, Wv, Wo = (np.asarray(w, dtype=np.float32) for w in (Wq, Wk, Wv, Wo))

    key = (B, S, DIM)
    if key not in _PROGRAM_CACHE:
        _PROGRAM_CACHE[key] = build_program(B, S, DIM)
    nc = _PROGRAM_CACHE[key]

    in_maps = make_in_maps(x, Wq, Wk, Wv, Wo)
    res = run_bass_kernel_spmd(
        nc, in_maps, core_ids=list(range(N_CORES)), trace=_trace
    )
    kernel.last_results = res
    acc = res.results[0]["out"].astype(np.float32)
    for c in range(1, N_CORES):
        acc = acc + res.results[c]["out"].astype(np.float32)
    # out is [16, 128, bs] oc-major; flatten to [dim, bs] then to [B, S, DIM]
    return np.ascontiguousarray(acc.reshape(DIM, B * S).T).reshape(B, S, DIM)


# revision 39
# speedup vs baseline: 1.0239x; 1.0239x over previous
"""Multi-head causal self-attention with RoPE, tensor-parallel over heads
across 8 Trainium2 NeuronCores.

Strategy (Megatron-style TP over heads), v4:
  - Each core owns 2 of the 16 heads: rows [c*256,(c+1)*256) of Wq/Wk/Wv
    and the matching columns of Wo. Host sums the 8 partial outputs.
  - q/k projections run in fp8e4m3 with DoubleRow perf mode (0.5
    cycles/row, 2x PE throughput); the host pre-scales Wq/Wk by 64 to
    center the fp8 dynamic range, and the 64*64 factor is folded into
    the exp scale. The q/k quantization noise washes out through the
    softmax (diffuse attention); the value path (v, Wo) stays bf16.
  - Everything else in bf16 (PSUM accumulation fp32).
  - All DRAM operands are host-prearranged so every DMA is contiguous
    per partition (128 descriptors, cheap HWDGE issue); out is written
    oc-major [16, 128, bs] and reassembled on host.
  - Batch-pipelined generator emission: p1(b0) -> [p1(b1) || attn(b0)]
    -> [attn(b1) || p3(ready slabs)] -> p3(last slab). Independent
    matmuls fill the in-order PE queue between each score-matmul ->
    exp -> AV-matmul dependency chain.
  - Causal: off-diagonal k-chunks full-width; the 4 diagonal chunks of
    each 512-wide q-chunk use a reduced column range plus a 128x128
    additive triangle mask.
"""

import sys

import numpy as np

B, S, DIM = 2, 2048, 2048
NUM_HEADS = 16
HD = 128
N_CORES = 8
HPC = NUM_HEADS // N_CORES  # heads per core
DLOC = HPC * HD             # per-core slice of the model dim
ROPE_BASE = 10000.0
NDIN = DIM // 128           # contraction chunks for projections
SC1 = 512                   # phase-1 s-chunk
QCH = 512                   # attention q-chunk
NQC = S // QCH              # q-chunks per batch
NS1B = S // SC1             # phase-1 s-chunks per batch
WSCALE = 64.0               # fp8 pre-scale on Wq/Wk

_PROGRAM_CACHE = {}


def _rope_tables_T(seq_len, head_dim):
    # match reference float32 arithmetic: inv_freq over even indices,
    # emb = cat(freqs, freqs); returned transposed [head_dim, seq_len]
    inv_freq = (
        1.0
        / (np.float32(ROPE_BASE)
           ** (np.arange(0, head_dim, 2, dtype=np.float32) / np.float32(head_dim)))
    ).astype(np.float32)
    t = np.arange(seq_len, dtype=np.float32)
    freqs = np.outer(t, inv_freq).astype(np.float32)      # [S, D/2]
    emb = np.concatenate([freqs, freqs], axis=-1)         # [S, D]
    return (
        np.ascontiguousarray(np.cos(emb).astype(np.float32).T),
        np.ascontiguousarray(np.sin(emb).astype(np.float32).T),
    )


def _rot_matrix_T(head_dim):
    # rotated = cat(-x[1::2], x[::2]) = R @ x; return R.T [D, D]
    d2 = head_dim // 2
    R = np.zeros((head_dim, head_dim), dtype=np.float32)
    for dp in range(d2):
        R[dp, 2 * dp + 1] = -1.0
    for dp in range(d2, head_dim):
        R[dp, 2 * (dp - d2)] = 1.0
    return np.ascontiguousarray(R.T)


def _tri_mask():
    # tri[kk, qq] = 0 if kk <= qq else -1e9 (additive, pre-exp-scale)
    kk = np.arange(128)[:, None]
    qq = np.arange(128)[None, :]
    return np.where(kk <= qq, 0.0, -1e9).astype(np.float32)


def _run_interleaved(*specs):
    """specs: (generator, steps_per_round). Round-robin until exhausted."""
    live = [[iter(g), k] for g, k in specs]
    while live:
        for item in list(live):
            g, k = item
            try:
                for _ in range(k):
                    next(g)
            except StopIteration:
                live.remove(item)


def build_program(b=B, s=S, dim=DIM):
    """Builds the per-core SPMD Bass program (identical on every core)."""
    if "/opt/trn_rl_repo" not in sys.path:
        sys.path.insert(0, "/opt/trn_rl_repo")
    import concourse.bacc as bacc
    import concourse.mybir as mybir
    import concourse.tile as tile

    f32 = mybir.dt.float32
    bf16 = mybir.dt.bfloat16
    fp8 = mybir.dt.float8e4
    EXP = mybir.ActivationFunctionType.Exp
    DR = mybir.MatmulPerfMode.DoubleRow

    bs = b * s
    scale = float(HD) ** -0.5 / (WSCALE * WSCALE)
    nsc = s // 128   # 128-token chunks per batch
    ngc = bs // SC1  # global 512-token chunks

    nc = bacc.Bacc("TRN2", target_bir_lowering=False, debug=False)

    # host-prearranged layouts: contiguous per partition
    xP_d = nc.dram_tensor("xP", [ngc, 128, NDIN * SC1], bf16, kind="ExternalInput")
    x8P_d = nc.dram_tensor("x8P", [ngc, 128, NDIN * SC1], fp8, kind="ExternalInput")
    wq8_d = nc.dram_tensor("wq8", [128, NDIN * DLOC], fp8, kind="ExternalInput")
    wk8_d = nc.dram_tensor("wk8", [128, NDIN * DLOC], fp8, kind="ExternalInput")
    wqb_d = nc.dram_tensor("wqb", [128, NDIN * DLOC], bf16, kind="ExternalInput")
    wkb_d = nc.dram_tensor("wkb", [128, NDIN * DLOC], bf16, kind="ExternalInput")
    wvP_d = nc.dram_tensor("wvP", [128, NDIN * DLOC], bf16, kind="ExternalInput")
    woP_d = nc.dram_tensor("woP", [128, HPC * dim], bf16, kind="ExternalInput")
    cosT_d = nc.dram_tensor("cosT", [HD, bs], bf16, kind="ExternalInput")
    sinT_d = nc.dram_tensor("sinT", [HD, bs], bf16, kind="ExternalInput")
    rT_d = nc.dram_tensor("rT", [HD, HD], bf16, kind="ExternalInput")
    ones_d = nc.dram_tensor("ones", [HD, HD], bf16, kind="ExternalInput")
    tri_d = nc.dram_tensor("tri", [HD, HD], bf16, kind="ExternalInput")
    out_d = nc.dram_tensor("out", [dim // 128, 128, bs], bf16, kind="ExternalOutput")

    with tile.TileContext(nc) as tc:
        with (
            tc.tile_pool(name="persist", bufs=1) as persist,
            tc.tile_pool(name="work", bufs=2) as work,
            tc.tile_pool(name="ptp", bufs=4) as ptp,
            tc.tile_pool(name="otp", bufs=3) as otp,
            tc.tile_pool(name="ps_at", bufs=1, space="PSUM") as ps_at,
        ):
            # ---- persistent tiles ----
            qT = persist.tile([128, HPC, bs], bf16)   # roped q (x64), [d, h, tok]
            kT = persist.tile([128, HPC, bs], bf16)
            vS = persist.tile([128, bs // 128, DLOC], bf16)  # [tok, chunk, d]
            uT = persist.tile([128, HPC, bs], bf16)   # attention out
            wq_s = persist.tile([128, NDIN, DLOC], fp8)
            wk_s = persist.tile([128, NDIN, DLOC], fp8)
            # bf16 copies for the first 512 tokens of each batch: early
            # causal rows have concentrated attention, so fp8 q/k noise
            # doesn't wash out there (x64-scaled like the fp8 path)
            wqb_s = persist.tile([128, NDIN, DLOC], bf16)
            wkb_s = persist.tile([128, NDIN, DLOC], bf16)
            wv_s = persist.tile([128, NDIN, DLOC], bf16)
            woT_s = persist.tile([128, HPC, dim], bf16)
            cosS = persist.tile([128, bs], bf16)
            sinS = persist.tile([128, bs], bf16)
            rTs = persist.tile([HD, HD], bf16)
            onesS = persist.tile([HD, HD], bf16)
            triS = persist.tile([HD, HD], bf16)

            # ---- startup DMAs: critical first, split across queues ----
            # first chain (si=0, bf16 path) needs wqb + xt interleaved in
            # consumption order on sync; everything else on scalar/gpsimd
            def wview(wd):
                return wd.rearrange("p (c m) -> p c m", c=NDIN)

            nc.sync.dma_start(out=wqb_s[:, 0:8, :], in_=wview(wqb_d)[:, 0:8, :])
            nc.scalar.dma_start(out=wkb_s, in_=wview(wkb_d))
            nc.gpsimd.dma_start(out=rTs, in_=rT_d[:])
            nc.gpsimd.dma_start(out=onesS, in_=ones_d[:])
            nc.gpsimd.dma_start(out=triS, in_=tri_d[:])

            def startup_x0(xt, xsrc):
                # split the first x chunk across the sync and gpsimd queues
                # so the first bf16 qk chain unblocks ~2x sooner
                nc.sync.dma_start(out=xt[:, 0:4, :], in_=xsrc[:, 0:4, :])
                nc.gpsimd.dma_start(out=xt[:, 8:12, :], in_=xsrc[:, 8:12, :])
                nc.sync.dma_start(out=wqb_s[:, 8:16, :], in_=wview(wqb_d)[:, 8:16, :])
                nc.sync.dma_start(out=xt[:, 4:8, :], in_=xsrc[:, 4:8, :])
                nc.gpsimd.dma_start(out=xt[:, 12:16, :], in_=xsrc[:, 12:16, :])

            def startup_rest():
                # nothing PE-side in seg1 depends on cos/sin (RoPE runs on
                # DVE and only feeds the qT/kT stores read in seg2), so the
                # fp8 weights jump the queue; wv rides the idle gpsimd SWDGE
                nc.gpsimd.dma_start(out=wv_s, in_=wview(wvP_d))
                nc.scalar.dma_start(out=wq_s, in_=wview(wq8_d))
                nc.scalar.dma_start(out=wk_s, in_=wview(wk8_d))
                nc.scalar.dma_start(out=cosS, in_=cosT_d[:])
                nc.scalar.dma_start(out=sinS, in_=sinT_d[:])
                # phase-3 weights: needed only after attn(b0); issue last
                nc.scalar.dma_start(
                    out=woT_s, in_=woP_d.rearrange("p (h n) -> p h n", h=HPC)
                )

            with (
                tc.tile_pool(name="p1x", bufs=2) as p1x,
                tc.tile_pool(name="p1ps", bufs=1, space="PSUM") as p1ps,
            ):
                # ------------- phase 1: qkv projections + RoPE -------------
                def p1_si(bi, si, first=False):
                    if True:
                        s0 = bi * s + si * SC1
                        g = s0 // SC1
                        xt = p1x.tile([128, NDIN, SC1], bf16, tag="xt")
                        xsrc = xP_d[g].rearrange("p (c t) -> p c t", c=NDIN)
                        x8src = x8P_d[g].rearrange("p (c t) -> p c t", c=NDIN)
                        xt8 = p1x.tile([128, NDIN, SC1], fp8, tag="xt8")
                        if first and si == 0:
                            startup_x0(xt, xsrc)
                            nc.sync.dma_start(out=xt8, in_=x8src)
                            startup_rest()
                        else:
                            # xt8 + half of xt on sync, other half on the
                            # scalar HWDGE (idle between startup and the
                            # first exps): a lone queue delivers ~260GB/s
                            # with 1KB lines and starves the fp8 chains
                            nc.sync.dma_start(out=xt8, in_=x8src)
                            nc.sync.dma_start(out=xt[:, :8, :], in_=xsrc[:, :8, :])
                            nc.scalar.dma_start(out=xt[:, 8:, :], in_=xsrc[:, 8:, :])
                        yield
                        vq = [0, 1, 2, 3]

                        def v_chain(sub):
                            vacc = p1ps.tile([128, DLOC], f32, tag="v", bufs=1)
                            for c in range(NDIN):
                                nc.tensor.matmul(
                                    vacc,
                                    lhsT=xt[:, c, sub * 128:(sub + 1) * 128],
                                    rhs=wv_s[:, c, :],
                                    start=(c == 0),
                                    stop=(c == NDIN - 1),
                                )
                                if c % 4 == 3:
                                    yield
                            nc.scalar.copy(vS[:, s0 // 128 + sub, :], vacc)
                            yield

                        ci = 0
                        HB = 256  # bf16 token boundary within si=0
                        for w8_t, wb_t, store in (
                            (wq_s, wqb_s, qT), (wk_s, wkb_s, kT)
                        ):
                            for h in range(HPC):
                                hsl = slice(h * HD, (h + 1) * HD)
                                acc = p1ps.tile([128, SC1], f32, tag="qk", bufs=2)
                                if si == 0:
                                    # tokens 0:256 in bf16 (concentrated
                                    # early-causal attention), 256:512 fp8
                                    for c in range(NDIN):
                                        nc.tensor.matmul(
                                            acc[:, 0:HB],
                                            lhsT=wb_t[:, c, hsl],
                                            rhs=xt[:, c, 0:HB],
                                            start=(c == 0),
                                            stop=(c == NDIN - 1),
                                        )
                                        if c % 4 == 3:
                                            yield
                                    for c2 in range(0, NDIN, 2):
                                        nc.tensor.matmul(
                                            acc[:, HB:],
                                            lhsT=w8_t[:, c2:c2 + 2, hsl],
                                            rhs=xt8[:, c2:c2 + 2, HB:],
                                            start=(c2 == 0),
                                            stop=(c2 == NDIN - 2),
                                            perf_mode=DR,
                                        )
                                        if c2 % 8 == 6:
                                            yield
                                else:
                                    for c2 in range(0, NDIN, 2):
                                        nc.tensor.matmul(
                                            acc,
                                            lhsT=w8_t[:, c2:c2 + 2, hsl],
                                            rhs=xt8[:, c2:c2 + 2, :],
                                            start=(c2 == 0),
                                            stop=(c2 == NDIN - 2),
                                            perf_mode=DR,
                                        )
                                        if c2 % 4 == 2:
                                            yield
                                raw = p1x.tile([128, SC1], bf16, tag="raw")
                                nc.vector.tensor_copy(raw, acc)
                                rot = p1ps.tile([128, SC1], f32, tag="rot", bufs=1)
                                nc.tensor.matmul(
                                    rot, lhsT=rTs, rhs=raw, start=True, stop=True
                                )
                                yield
                                t1 = p1x.tile([128, SC1], bf16, tag="t1")
                                nc.vector.tensor_mul(t1, raw, cosS[:, s0:s0 + SC1])
                                t2 = p1x.tile([128, SC1], bf16, tag="t2")
                                nc.vector.tensor_mul(t2, rot, sinS[:, s0:s0 + SC1])
                                nc.vector.tensor_add(
                                    store[:, h, s0:s0 + SC1], t1, t2
                                )
                                yield
                                ci += 1
                                # delay v chains past the first two qk chains
                                # (wv lands late in the startup order)
                                if vq and ci >= 2:
                                    yield from v_chain(vq.pop(0))
                        while vq:
                            yield from v_chain(vq.pop(0))

                # ------------- phase 2: causal attention -------------
                # p3q: output-projection slabs whose uT columns are complete
                p3q = []

                def attn_gen(bi, qlo=0, qhi=NQC):
                    # NOTE: pt must stay bf16 — real scores reach 9.08, so
                    # exp hits 8.8e3, far over fp8-e4m3's 448 max (NaN)
                    for qc in range(qlo, qhi):
                        for h in range(HPC):
                            q0 = bi * s + qc * QCH
                            nkc = (qc + 1) * QCH // 128
                            outp = ps_at.tile([128, QCH], f32, tag="o", bufs=1)
                            lrep = ps_at.tile([128, QCH], f32, tag="l", bufs=1)
                            npair = (nkc - 4) // 2
                            nones = npair + 4
                            li = 0
                            prev_pt = None
                            for kc in range(nkc):
                                k0 = bi * s + kc * 128
                                di = kc - (nkc - 4)
                                lo = max(0, di * 128)
                                st = ps_at.tile([128, QCH], f32, tag="st", bufs=2)
                                nc.tensor.matmul(
                                    st[:, lo:],
                                    lhsT=kT[:, h, k0:k0 + 128],
                                    rhs=qT[:, h, q0 + lo:q0 + QCH],
                                    start=True,
                                    stop=True,
                                )
                                if di >= 0:
                                    nc.vector.tensor_add(
                                        st[:, lo:lo + 128], st[:, lo:lo + 128], triS
                                    )
                                pt = ptp.tile([128, QCH], bf16, tag="pt")
                                nc.scalar.activation(
                                    pt[:, lo:], st[:, lo:], EXP, scale=scale
                                )
                                yield
                                nc.tensor.matmul(
                                    outp[:, lo:],
                                    lhsT=vS[:, bi * nsc + kc, h * HD:(h + 1) * HD],
                                    rhs=pt[:, lo:],
                                    start=(kc == 0),
                                    stop=(kc == nkc - 1),
                                )
                                if di >= 0:
                                    # diagonal chunks: single range-reduced
                                    # denominator matmul
                                    nc.tensor.matmul(
                                        lrep[:, lo:],
                                        lhsT=onesS,
                                        rhs=pt[:, lo:],
                                        start=(li == 0),
                                        stop=(li == nones - 1),
                                    )
                                    li += 1
                                elif kc % 2 == 1:
                                    # off-diagonal: pair-sum on DVE/GpSimd,
                                    # then one denominator matmul per pair
                                    lp = work.tile([128, QCH], bf16, tag="lp")
                                    eng = nc.vector if li % 2 == 0 else nc.gpsimd
                                    eng.tensor_add(lp, prev_pt, pt)
                                    nc.tensor.matmul(
                                        lrep,
                                        lhsT=onesS,
                                        rhs=lp,
                                        start=(li == 0),
                                        stop=(li == nones - 1),
                                    )
                                    li += 1
                                prev_pt = pt
                                yield
                            rec = work.tile([128, QCH], f32, tag="rec")
                            nc.vector.reciprocal_approx_fast(rec, lrep)
                            nc.vector.tensor_mul(
                                uT[:, h, q0:q0 + QCH], outp, rec
                            )
                            yield
                        p3q.append((bi, qc))

                from itertools import chain

                # seg1a: first 512-token chunk of batch 0 alone
                _run_interleaved((p1_si(0, 0, first=True), 1))
                # seg1b: rest of p1(b0); attention(b0, qc0) fills the
                # x-supply-bound gaps (its operands landed with si0)
                _run_interleaved(
                    (chain(*(p1_si(0, si) for si in range(1, NS1B))), 1),
                    (attn_gen(0, 0, 1), 1),
                )
                # seg2: projections(b1) interleaved with attention(b0, qc1-3)
                _run_interleaved(
                    (chain(*(p1_si(1, si) for si in range(NS1B))), 1),
                    (attn_gen(0, 1, NQC), 1),
                )

            # p1 PSUM pools closed; banks free for phase 3
            with tc.tile_pool(name="ps3", bufs=2, space="PSUM") as ps3:
                # ------------- phase 3: output projection -------------
                # consumes ready 512-column slabs from p3q; a slab (bi, qc)
                # is pushed once both heads of that q-chunk wrote uT

                def p3_slab(bi, qc, ceng):
                    c0 = bi * s + qc * QCH
                    for oc in range(dim // 128):
                        pos = ps3.tile([128, QCH], f32, tag="p3", bufs=4)
                        for h in range(HPC):
                            nc.tensor.matmul(
                                pos,
                                lhsT=woT_s[:, h, oc * 128:(oc + 1) * 128],
                                rhs=uT[:, h, c0:c0 + QCH],
                                start=(h == 0),
                                stop=(h == HPC - 1),
                            )
                        ot = otp.tile([128, QCH], bf16, tag="ot", bufs=6)
                        eng = ceng[oc % len(ceng)]
                        if eng is nc.scalar:
                            eng.copy(ot, pos)
                        else:
                            eng.tensor_copy(ot, pos)
                        yield
                        # split write issue across the sync and gpsimd queues
                        deng = nc.sync if oc % 2 == 0 else nc.gpsimd
                        deng.dma_start(out=out_d[oc, :, c0:c0 + QCH], in_=ot)
                        yield

                def p3_consumer(n_slabs, ceng):
                    served = 0
                    while served < n_slabs:
                        if not p3q:
                            yield  # waiting on attention progress
                            continue
                        bi, qc = p3q.pop(0)
                        yield from p3_slab(bi, qc, ceng)
                        served += 1

                # seg3: attention(b1) interleaved with all ready output-
                # projection slabs (b0 now, b1 as q-chunks complete);
                # GpSimd cannot access PSUM, and Act owns the exps, so
                # copies lean on DVE
                _run_interleaved(
                    (attn_gen(1), 1),
                    (p3_consumer(2 * NQC - 1, [nc.vector, nc.vector, nc.scalar]), 1),
                )
                # tail: the final slab has nothing left to overlap with;
                # alternate copies over the now-idle Act and DVE
                _run_interleaved((p3_consumer(1, [nc.scalar, nc.vector]), 1))

    nc.compile()
    return nc


def make_in_maps(x, Wq, Wk, Wv, Wo, b=B, s=S, dim=DIM, n_cores=N_CORES):
    import ml_dtypes

    bf = ml_dtypes.bfloat16
    f8 = ml_dtypes.float8_e4m3fn
    bs = b * s
    xT = np.ascontiguousarray(x.reshape(bs, dim).T)          # [dim, bs] f32
    # [g, p, c*SC1+t]: per-512-token chunk, contiguous per partition
    xP4 = np.ascontiguousarray(
        xT.reshape(NDIN, 128, bs // SC1, SC1).transpose(2, 1, 0, 3)
    ).reshape(bs // SC1, 128, NDIN * SC1)
    xP = xP4.astype(bf)
    x8P = xP4.astype(f8)
    cosT1, sinT1 = _rope_tables_T(s, HD)
    cosT = np.ascontiguousarray(np.tile(cosT1, (1, b))).astype(bf)
    sinT = np.ascontiguousarray(np.tile(sinT1, (1, b))).astype(bf)
    rT = _rot_matrix_T(HD).astype(bf)
    ones = np.ones((HD, HD), dtype=np.float32).astype(bf)
    tri = _tri_mask().astype(bf)

    def wprep(wT):  # [dim, DLOC] -> [128, NDIN*DLOC] contiguous rows
        return np.ascontiguousarray(
            wT.reshape(NDIN, 128, DLOC).transpose(1, 0, 2)
        ).reshape(128, NDIN * DLOC)

    in_maps = []
    for c in range(n_cores):
        sl = slice(c * DLOC, (c + 1) * DLOC)
        woT = np.ascontiguousarray(Wo[:, sl].T)  # [DLOC, dim]
        woP = np.ascontiguousarray(
            woT.reshape(HPC, 128, dim).transpose(1, 0, 2)
        ).reshape(128, HPC * dim)
        in_maps.append(
            {
                "xP": xP,
                "x8P": x8P,
                "wq8": wprep(
                    np.ascontiguousarray(Wq[sl, :].T) * WSCALE
                ).astype(f8),
                "wk8": wprep(
                    np.ascontiguousarray(Wk[sl, :].T) * WSCALE
                ).astype(f8),
                "wqb": wprep(
                    np.ascontiguousarray(Wq[sl, :].T) * WSCALE
                ).astype(bf),
                "wkb": wprep(
                    np.ascontiguousarray(Wk[sl, :].T) * WSCALE
                ).astype(bf),
                "wvP": wprep(np.ascontiguousarray(Wv[sl, :].T)).astype(bf),
                "woP": woP.astype(bf),
                "cosT": cosT,
                "sinT": sinT,
                "rT": rT,
                "ones": ones,
                "tri": tri,
            }
        )
    return in_maps


def kernel(x, Wq, Wk, Wv, Wo, _trace=False):
    """Full-input / full-output entry point. Shards over 8 cores internally."""
    if "/opt/trn_rl_repo" not in sys.path:
        sys.path.insert(0, "/opt/trn_rl_repo")
    from concourse.bass_utils import run_bass_kernel_spmd

    x = np.asarray(x, dtype=np.float32)
    Wq, Wk, Wv, Wo = (np.asarray(w, dtype=np.float32) for w in (Wq, Wk, Wv, Wo))

    key = (B, S, DIM)
    if key not in _PROGRAM_CACHE:
        _PROGRAM_CACHE[key] = build_program(B, S, DIM)
    nc = _PROGRAM_CACHE[key]

    in_maps = make_in_maps(x, Wq, Wk, Wv, Wo)
    res = run_bass_kernel_spmd(
        nc, in_maps, core_ids=list(range(N_CORES)), trace=_trace
    )
    kernel.last_results = res
    acc = res.results[0]["out"].astype(np.float32)
    for c in range(1, N_CORES):
        acc = acc + res.results[c]["out"].astype(np.float32)
    # out is [16, 128, bs] oc-major; flatten to [dim, bs] then to [B, S, DIM]
    return np.ascontiguousarray(acc.reshape(DIM, B * S).T).reshape(B, S, DIM)


# revision 40
# speedup vs baseline: 1.0270x; 1.0030x over previous
"""Multi-head causal self-attention with RoPE, tensor-parallel over heads
across 8 Trainium2 NeuronCores.

Strategy (Megatron-style TP over heads):
  - Each core owns 2 of the 16 heads: rows [c*256,(c+1)*256) of Wq/Wk/Wv
    and the matching columns of Wo. Host sums the 8 partial outputs.
  - q/k projections run in fp8e4m3 with DoubleRow perf mode (~4x bf16
    rate per k-chunk pair on the PE) for all tokens except the first
    256 of each batch, which stay bf16: early causal rows attend over
    few keys, so fp8 q/k noise does not wash out through the softmax
    there. The host pre-scales Wq/Wk by 64 to center the fp8 dynamic
    range; the 64*64 factor is folded into the exp scale.
  - Everything else in bf16 (PSUM accumulation fp32). Attention
    weights must stay bf16: real scores reach 9.1, so raw exp (8.8e3)
    overflows fp8-e4m3 (max 448, NaN on overflow).
  - All DRAM operands are host-prearranged so every DMA is contiguous
    per partition (cheap HWDGE issue); out is written oc-major
    [16, 128, bs] and reassembled on host.
  - Batch-pipelined generator emission: p1(b0) -> [p1(b1) || attn(b0)]
    -> [attn(b1) || p3(ready 512-col slabs)] -> p3(last slab).
    Independent matmuls fill the in-order PE queue between each
    score-matmul -> exp -> AV-matmul dependency chain, hiding the
    Scalar-engine exp latency that bounded the original kernel.
  - Causal: off-diagonal k-chunks full-width; the 4 diagonal chunks of
    each 512-wide q-chunk use a reduced column range plus a 128x128
    additive triangle mask. Softmax denominators come from ones-matmuls
    (pair-summed on DVE/GpSimd for off-diagonal chunks).
"""

import sys

import numpy as np

B, S, DIM = 2, 2048, 2048
NUM_HEADS = 16
HD = 128
N_CORES = 8
HPC = NUM_HEADS // N_CORES  # heads per core
DLOC = HPC * HD             # per-core slice of the model dim
ROPE_BASE = 10000.0
NDIN = DIM // 128           # contraction chunks for projections
SC1 = 512                   # phase-1 s-chunk
QCH = 512                   # attention q-chunk
NQC = S // QCH              # q-chunks per batch
NS1B = S // SC1             # phase-1 s-chunks per batch
WSCALE = 64.0               # fp8 pre-scale on Wq/Wk

_PROGRAM_CACHE = {}


def _rope_tables_T(seq_len, head_dim):
    # match reference float32 arithmetic: inv_freq over even indices,
    # emb = cat(freqs, freqs); returned transposed [head_dim, seq_len]
    inv_freq = (
        1.0
        / (np.float32(ROPE_BASE)
           ** (np.arange(0, head_dim, 2, dtype=np.float32) / np.float32(head_dim)))
    ).astype(np.float32)
    t = np.arange(seq_len, dtype=np.float32)
    freqs = np.outer(t, inv_freq).astype(np.float32)      # [S, D/2]
    emb = np.concatenate([freqs, freqs], axis=-1)         # [S, D]
    return (
        np.ascontiguousarray(np.cos(emb).astype(np.float32).T),
        np.ascontiguousarray(np.sin(emb).astype(np.float32).T),
    )


def _rot_matrix_T(head_dim):
    # rotated = cat(-x[1::2], x[::2]) = R @ x; return R.T [D, D]
    d2 = head_dim // 2
    R = np.zeros((head_dim, head_dim), dtype=np.float32)
    for dp in range(d2):
        R[dp, 2 * dp + 1] = -1.0
    for dp in range(d2, head_dim):
        R[dp, 2 * (dp - d2)] = 1.0
    return np.ascontiguousarray(R.T)


def _tri_mask():
    # tri[kk, qq] = 0 if kk <= qq else -1e9 (additive, pre-exp-scale)
    kk = np.arange(128)[:, None]
    qq = np.arange(128)[None, :]
    return np.where(kk <= qq, 0.0, -1e9).astype(np.float32)


def _run_interleaved(*specs):
    """specs: (generator, steps_per_round). Round-robin until exhausted."""
    live = [[iter(g), k] for g, k in specs]
    while live:
        for item in list(live):
            g, k = item
            try:
                for _ in range(k):
                    next(g)
            except StopIteration:
                live.remove(item)


def build_program(b=B, s=S, dim=DIM):
    """Builds the per-core SPMD Bass program (identical on every core)."""
    if "/opt/trn_rl_repo" not in sys.path:
        sys.path.insert(0, "/opt/trn_rl_repo")
    import concourse.bacc as bacc
    import concourse.mybir as mybir
    import concourse.tile as tile

    f32 = mybir.dt.float32
    bf16 = mybir.dt.bfloat16
    fp8 = mybir.dt.float8e4
    EXP = mybir.ActivationFunctionType.Exp
    DR = mybir.MatmulPerfMode.DoubleRow

    bs = b * s
    scale = float(HD) ** -0.5 / (WSCALE * WSCALE)
    nsc = s // 128   # 128-token chunks per batch
    ngc = bs // SC1  # global 512-token chunks

    nc = bacc.Bacc("TRN2", target_bir_lowering=False, debug=False)

    # host-prearranged layouts: contiguous per partition
    xP_d = nc.dram_tensor("xP", [ngc, 128, NDIN * SC1], bf16, kind="ExternalInput")
    x8P_d = nc.dram_tensor("x8P", [ngc, 128, NDIN * SC1], fp8, kind="ExternalInput")
    wq8_d = nc.dram_tensor("wq8", [128, NDIN * DLOC], fp8, kind="ExternalInput")
    wk8_d = nc.dram_tensor("wk8", [128, NDIN * DLOC], fp8, kind="ExternalInput")
    wqb_d = nc.dram_tensor("wqb", [128, NDIN * DLOC], bf16, kind="ExternalInput")
    wkb_d = nc.dram_tensor("wkb", [128, NDIN * DLOC], bf16, kind="ExternalInput")
    wvP_d = nc.dram_tensor("wvP", [128, NDIN * DLOC], bf16, kind="ExternalInput")
    woP_d = nc.dram_tensor("woP", [128, HPC * dim], bf16, kind="ExternalInput")
    cosT_d = nc.dram_tensor("cosT", [HD, bs], bf16, kind="ExternalInput")
    sinT_d = nc.dram_tensor("sinT", [HD, bs], bf16, kind="ExternalInput")
    rT_d = nc.dram_tensor("rT", [HD, HD], bf16, kind="ExternalInput")
    ones_d = nc.dram_tensor("ones", [HD, HD], bf16, kind="ExternalInput")
    tri_d = nc.dram_tensor("tri", [HD, HD], bf16, kind="ExternalInput")
    out_d = nc.dram_tensor("out", [dim // 128, 128, bs], bf16, kind="ExternalOutput")

    with tile.TileContext(nc) as tc:
        with (
            tc.tile_pool(name="persist", bufs=1) as persist,
            tc.tile_pool(name="work", bufs=2) as work,
            tc.tile_pool(name="ptp", bufs=4) as ptp,
            tc.tile_pool(name="otp", bufs=3) as otp,
            tc.tile_pool(name="ps_at", bufs=1, space="PSUM") as ps_at,
        ):
            # ---- persistent tiles ----
            qT = persist.tile([128, HPC, bs], bf16)   # roped q (x64), [d, h, tok]
            kT = persist.tile([128, HPC, bs], bf16)
            vS = persist.tile([128, bs // 128, DLOC], bf16)  # [tok, chunk, d]
            uT = persist.tile([128, HPC, bs], bf16)   # attention out
            wq_s = persist.tile([128, NDIN, DLOC], fp8)
            wk_s = persist.tile([128, NDIN, DLOC], fp8)
            # bf16 copies for the first 512 tokens of each batch: early
            # causal rows have concentrated attention, so fp8 q/k noise
            # doesn't wash out there (x64-scaled like the fp8 path)
            wqb_s = persist.tile([128, NDIN, DLOC], bf16)
            wkb_s = persist.tile([128, NDIN, DLOC], bf16)
            wv_s = persist.tile([128, NDIN, DLOC], bf16)
            woT_s = persist.tile([128, HPC, dim], bf16)
            cosS = persist.tile([128, bs], bf16)
            sinS = persist.tile([128, bs], bf16)
            rTs = persist.tile([HD, HD], bf16)
            onesS = persist.tile([HD, HD], bf16)
            triS = persist.tile([HD, HD], bf16)

            # ---- startup DMAs: critical first, split across queues ----
            # first chain (si=0, bf16 path) needs wqb + xt interleaved in
            # consumption order on sync; everything else on scalar/gpsimd
            def wview(wd):
                return wd.rearrange("p (c m) -> p c m", c=NDIN)

            nc.sync.dma_start(out=wqb_s[:, 0:8, :], in_=wview(wqb_d)[:, 0:8, :])
            nc.scalar.dma_start(out=wkb_s, in_=wview(wkb_d))
            nc.gpsimd.dma_start(out=rTs, in_=rT_d[:])
            nc.gpsimd.dma_start(out=onesS, in_=ones_d[:])
            nc.gpsimd.dma_start(out=triS, in_=tri_d[:])

            def startup_x0(xt, xsrc):
                # split the first x chunk across the sync and gpsimd queues
                # so the first bf16 qk chain unblocks ~2x sooner
                nc.sync.dma_start(out=xt[:, 0:4, :], in_=xsrc[:, 0:4, :])
                nc.gpsimd.dma_start(out=xt[:, 8:12, :], in_=xsrc[:, 8:12, :])
                nc.sync.dma_start(out=wqb_s[:, 8:16, :], in_=wview(wqb_d)[:, 8:16, :])
                nc.sync.dma_start(out=xt[:, 4:8, :], in_=xsrc[:, 4:8, :])
                nc.gpsimd.dma_start(out=xt[:, 12:16, :], in_=xsrc[:, 12:16, :])

            def startup_rest():
                # nothing PE-side in seg1 depends on cos/sin (RoPE runs on
                # DVE and only feeds the qT/kT stores read in seg2), so the
                # fp8 weights jump the queue; wv rides the idle gpsimd SWDGE
                nc.gpsimd.dma_start(out=wv_s, in_=wview(wvP_d))
                nc.scalar.dma_start(out=wq_s, in_=wview(wq8_d))
                nc.scalar.dma_start(out=wk_s, in_=wview(wk8_d))
                nc.scalar.dma_start(out=cosS, in_=cosT_d[:])
                nc.scalar.dma_start(out=sinS, in_=sinT_d[:])
                # phase-3 weights: needed only after attn(b0); issue last
                nc.scalar.dma_start(
                    out=woT_s, in_=woP_d.rearrange("p (h n) -> p h n", h=HPC)
                )

            with (
                tc.tile_pool(name="p1x", bufs=2) as p1x,
                tc.tile_pool(name="p1ps", bufs=1, space="PSUM") as p1ps,
            ):
                # ------------- phase 1: qkv projections + RoPE -------------
                def p1_gen(bi, first=False):
                    for si in range(NS1B):
                        s0 = bi * s + si * SC1
                        g = s0 // SC1
                        xt = p1x.tile([128, NDIN, SC1], bf16, tag="xt")
                        xsrc = xP_d[g].rearrange("p (c t) -> p c t", c=NDIN)
                        x8src = x8P_d[g].rearrange("p (c t) -> p c t", c=NDIN)
                        xt8 = p1x.tile([128, NDIN, SC1], fp8, tag="xt8")
                        if first and si == 0:
                            startup_x0(xt, xsrc)
                            nc.sync.dma_start(out=xt8, in_=x8src)
                            startup_rest()
                        else:
                            nc.sync.dma_start(out=xt8, in_=x8src)
                            nc.sync.dma_start(out=xt[:, :8, :], in_=xsrc[:, :8, :])
                            nc.sync.dma_start(out=xt[:, 8:, :], in_=xsrc[:, 8:, :])
                        yield
                        vq = [0, 1, 2, 3]

                        def v_chain(sub):
                            vacc = p1ps.tile([128, DLOC], f32, tag="v", bufs=1)
                            for c in range(NDIN):
                                nc.tensor.matmul(
                                    vacc,
                                    lhsT=xt[:, c, sub * 128:(sub + 1) * 128],
                                    rhs=wv_s[:, c, :],
                                    start=(c == 0),
                                    stop=(c == NDIN - 1),
                                )
                                if c % 4 == 3:
                                    yield
                            nc.scalar.copy(vS[:, s0 // 128 + sub, :], vacc)
                            yield

                        ci = 0
                        HB = 256  # bf16 token boundary within si=0
                        for w8_t, wb_t, store in (
                            (wq_s, wqb_s, qT), (wk_s, wkb_s, kT)
                        ):
                            for h in range(HPC):
                                hsl = slice(h * HD, (h + 1) * HD)
                                acc = p1ps.tile([128, SC1], f32, tag="qk", bufs=2)
                                if si == 0:
                                    # tokens 0:256 in bf16 (concentrated
                                    # early-causal attention), 256:512 fp8
                                    for c in range(NDIN):
                                        nc.tensor.matmul(
                                            acc[:, 0:HB],
                                            lhsT=wb_t[:, c, hsl],
                                            rhs=xt[:, c, 0:HB],
                                            start=(c == 0),
                                            stop=(c == NDIN - 1),
                                        )
                                        if c % 4 == 3:
                                            yield
                                    for c2 in range(0, NDIN, 2):
                                        nc.tensor.matmul(
                                            acc[:, HB:],
                                            lhsT=w8_t[:, c2:c2 + 2, hsl],
                                            rhs=xt8[:, c2:c2 + 2, HB:],
                                            start=(c2 == 0),
                                            stop=(c2 == NDIN - 2),
                                            perf_mode=DR,
                                        )
                                        if c2 % 8 == 6:
                                            yield
                                else:
                                    for c2 in range(0, NDIN, 2):
                                        nc.tensor.matmul(
                                            acc,
                                            lhsT=w8_t[:, c2:c2 + 2, hsl],
                                            rhs=xt8[:, c2:c2 + 2, :],
                                            start=(c2 == 0),
                                            stop=(c2 == NDIN - 2),
                                            perf_mode=DR,
                                        )
                                        if c2 % 4 == 2:
                                            yield
                                raw = p1x.tile([128, SC1], bf16, tag="raw")
                                nc.vector.tensor_copy(raw, acc)
                                rot = p1ps.tile([128, SC1], f32, tag="rot", bufs=1)
                                nc.tensor.matmul(
                                    rot, lhsT=rTs, rhs=raw, start=True, stop=True
                                )
                                yield
                                t1 = p1x.tile([128, SC1], bf16, tag="t1")
                                nc.vector.tensor_mul(t1, raw, cosS[:, s0:s0 + SC1])
                                t2 = p1x.tile([128, SC1], bf16, tag="t2")
                                nc.vector.tensor_mul(t2, rot, sinS[:, s0:s0 + SC1])
                                nc.vector.tensor_add(
                                    store[:, h, s0:s0 + SC1], t1, t2
                                )
                                yield
                                ci += 1
                                # delay v chains past the first two qk chains
                                # (wv lands late in the startup order)
                                if vq and ci >= 2:
                                    yield from v_chain(vq.pop(0))
                        while vq:
                            yield from v_chain(vq.pop(0))

                # ------------- phase 2: causal attention -------------
                # p3q: output-projection slabs whose uT columns are complete
                p3q = []

                def attn_gen(bi):
                    # NOTE: pt must stay bf16 — real scores reach 9.08, so
                    # exp hits 8.8e3, far over fp8-e4m3's 448 max (NaN)
                    for qc in range(NQC):
                        for h in range(HPC):
                            q0 = bi * s + qc * QCH
                            nkc = (qc + 1) * QCH // 128
                            outp = ps_at.tile([128, QCH], f32, tag="o", bufs=1)
                            lrep = ps_at.tile([128, QCH], f32, tag="l", bufs=1)
                            npair = (nkc - 4) // 2
                            nones = npair + 4
                            li = 0
                            prev_pt = None
                            for kc in range(nkc):
                                k0 = bi * s + kc * 128
                                di = kc - (nkc - 4)
                                lo = max(0, di * 128)
                                st = ps_at.tile([128, QCH], f32, tag="st", bufs=2)
                                nc.tensor.matmul(
                                    st[:, lo:],
                                    lhsT=kT[:, h, k0:k0 + 128],
                                    rhs=qT[:, h, q0 + lo:q0 + QCH],
                                    start=True,
                                    stop=True,
                                )
                                if di >= 0:
                                    nc.vector.tensor_add(
                                        st[:, lo:lo + 128], st[:, lo:lo + 128], triS
                                    )
                                pt = ptp.tile([128, QCH], bf16, tag="pt")
                                nc.scalar.activation(
                                    pt[:, lo:], st[:, lo:], EXP, scale=scale
                                )
                                yield
                                nc.tensor.matmul(
                                    outp[:, lo:],
                                    lhsT=vS[:, bi * nsc + kc, h * HD:(h + 1) * HD],
                                    rhs=pt[:, lo:],
                                    start=(kc == 0),
                                    stop=(kc == nkc - 1),
                                )
                                if di >= 0:
                                    # diagonal chunks: single range-reduced
                                    # denominator matmul
                                    nc.tensor.matmul(
                                        lrep[:, lo:],
                                        lhsT=onesS,
                                        rhs=pt[:, lo:],
                                        start=(li == 0),
                                        stop=(li == nones - 1),
                                    )
                                    li += 1
                                elif kc % 2 == 1:
                                    # off-diagonal: pair-sum on DVE/GpSimd,
                                    # then one denominator matmul per pair
                                    lp = work.tile([128, QCH], bf16, tag="lp")
                                    eng = nc.vector if li % 2 == 0 else nc.gpsimd
                                    eng.tensor_add(lp, prev_pt, pt)
                                    nc.tensor.matmul(
                                        lrep,
                                        lhsT=onesS,
                                        rhs=lp,
                                        start=(li == 0),
                                        stop=(li == nones - 1),
                                    )
                                    li += 1
                                prev_pt = pt
                                yield
                            rec = work.tile([128, QCH], f32, tag="rec")
                            nc.vector.reciprocal_approx_fast(rec, lrep)
                            nc.vector.tensor_mul(
                                uT[:, h, q0:q0 + QCH], outp, rec
                            )
                            yield
                        p3q.append((bi, qc))

                # seg1: projections for batch 0 alone
                _run_interleaved((p1_gen(0, first=True), 1))
                # seg2: projections(b1) interleaved with attention(b0)
                _run_interleaved((p1_gen(1), 1), (attn_gen(0), 1))

            # p1 PSUM pools closed; banks free for phase 3
            with tc.tile_pool(name="ps3", bufs=2, space="PSUM") as ps3:
                # ------------- phase 3: output projection -------------
                # consumes ready 512-column slabs from p3q; a slab (bi, qc)
                # is pushed once both heads of that q-chunk wrote uT

                def p3_slab(bi, qc, ceng):
                    c0 = bi * s + qc * QCH
                    for oc in range(dim // 128):
                        pos = ps3.tile([128, QCH], f32, tag="p3", bufs=4)
                        for h in range(HPC):
                            nc.tensor.matmul(
                                pos,
                                lhsT=woT_s[:, h, oc * 128:(oc + 1) * 128],
                                rhs=uT[:, h, c0:c0 + QCH],
                                start=(h == 0),
                                stop=(h == HPC - 1),
                            )
                        ot = otp.tile([128, QCH], bf16, tag="ot", bufs=6)
                        eng = ceng[oc % len(ceng)]
                        if eng is nc.scalar:
                            eng.copy(ot, pos)
                        else:
                            eng.tensor_copy(ot, pos)
                        yield
                        # split write issue across the sync and gpsimd queues
                        deng = nc.sync if oc % 2 == 0 else nc.gpsimd
                        deng.dma_start(out=out_d[oc, :, c0:c0 + QCH], in_=ot)
                        yield

                def p3_consumer(n_slabs, ceng):
                    served = 0
                    while served < n_slabs:
                        if not p3q:
                            yield  # waiting on attention progress
                            continue
                        bi, qc = p3q.pop(0)
                        yield from p3_slab(bi, qc, ceng)
                        served += 1

                # seg3: attention(b1) interleaved with all ready output-
                # projection slabs (b0 now, b1 as q-chunks complete);
                # GpSimd cannot access PSUM, and Act owns the exps, so
                # copies lean on DVE
                _run_interleaved(
                    (attn_gen(1), 1),
                    (p3_consumer(2 * NQC - 1, [nc.vector, nc.vector, nc.scalar]), 1),
                )
                # tail: the final slab has nothing left to overlap with;
                # alternate copies over the now-idle Act and DVE
                _run_interleaved((p3_consumer(1, [nc.scalar, nc.vector]), 1))

    nc.compile()
    return nc


def make_in_maps(x, Wq, Wk, Wv, Wo, b=B, s=S, dim=DIM, n_cores=N_CORES):
    import ml_dtypes

    bf = ml_dtypes.bfloat16
    f8 = ml_dtypes.float8_e4m3fn
    bs = b * s
    xT = np.ascontiguousarray(x.reshape(bs, dim).T)          # [dim, bs] f32
    # [g, p, c*SC1+t]: per-512-token chunk, contiguous per partition
    xP4 = np.ascontiguousarray(
        xT.reshape(NDIN, 128, bs // SC1, SC1).transpose(2, 1, 0, 3)
    ).reshape(bs // SC1, 128, NDIN * SC1)
    xP = xP4.astype(bf)
    x8P = xP4.astype(f8)
    cosT1, sinT1 = _rope_tables_T(s, HD)
    cosT = np.ascontiguousarray(np.tile(cosT1, (1, b))).astype(bf)
    sinT = np.ascontiguousarray(np.tile(sinT1, (1, b))).astype(bf)
    rT = _rot_matrix_T(HD).astype(bf)
    ones = np.ones((HD, HD), dtype=np.float32).astype(bf)
    tri = _tri_mask().astype(bf)

    def wprep(wT):  # [dim, DLOC] -> [128, NDIN*DLOC] contiguous rows
        return np.ascontiguousarray(
            wT.reshape(NDIN, 128, DLOC).transpose(1, 0, 2)
        ).reshape(128, NDIN * DLOC)

    in_maps = []
    for c in range(n_cores):
        sl = slice(c * DLOC, (c + 1) * DLOC)
        woT = np.ascontiguousarray(Wo[:, sl].T)  # [DLOC, dim]
        woP = np.ascontiguousarray(
            woT.reshape(HPC, 128, dim).transpose(1, 0, 2)
        ).reshape(128, HPC * dim)
        in_maps.append(
            {
                "xP": xP,
                "x8P": x8P,
                "wq8": wprep(
                    np.ascontiguousarray(Wq[sl, :].T) * WSCALE
                ).astype(f8),
                "wk8": wprep(
                    np.ascontiguousarray(Wk[sl, :].T) * WSCALE
                ).astype(f8),
                "wqb": wprep(
                    np.ascontiguousarray(Wq[sl, :].T) * WSCALE
                ).astype(bf),
                "wkb": wprep(
                    np.ascontiguousarray(Wk[sl, :].T) * WSCALE
                ).astype(bf),
                "wvP": wprep(np.ascontiguousarray(Wv[sl, :].T)).astype(bf),
                "woP": woP.astype(bf),
                "cosT": cosT,
                "sinT": sinT,
                "rT": rT,
                "ones": ones,
                "tri": tri,
            }
        )
    return in_maps


def kernel(x, Wq, Wk, Wv, Wo, _trace=False):
    """Full-input / full-output entry point. Shards over 8 cores internally."""
    if "/opt/trn_rl_repo" not in sys.path:
        sys.path.insert(0, "/opt/trn_rl_repo")
    from concourse.bass_utils import run_bass_kernel_spmd

    x = np.asarray(x, dtype=np.float32)
    Wq, Wk, Wv, Wo = (np.asarray(w, dtype=np.float32) for w in (Wq, Wk, Wv, Wo))

    key = (B, S, DIM)
    if key not in _PROGRAM_CACHE:
        _PROGRAM_CACHE[key] = build_program(B, S, DIM)
    nc = _PROGRAM_CACHE[key]

    in_maps = make_in_maps(x, Wq, Wk, Wv, Wo)
    res = run_bass_kernel_spmd(
        nc, in_maps, core_ids=list(range(N_CORES)), trace=_trace
    )
    kernel.last_results = res
    acc = res.results[0]["out"].astype(np.float32)
    for c in range(1, N_CORES):
        acc = acc + res.results[c]["out"].astype(np.float32)
    # out is [16, 128, bs] oc-major; flatten to [dim, bs] then to [B, S, DIM]
    return np.ascontiguousarray(acc.reshape(DIM, B * S).T).reshape(B, S, DIM)
